# revision 24
# baseline (speedup 1.0000x reference)
"""BitLinear (ternary-weight / int8-activation quantized linear) on 8 TRN2 NeuronCores.

Computation (matches reference):
    w_scale = mean(|W|, axis=in) + eps            # [out, 1]
    w_quant = clip(round(W / w_scale), -1, 1)     # ternary
    a_scale = max(|x|, axis=in) + eps             # per token
    a_quant = round(x / a_scale * 127)            # int8 range
    y       = (a_quant @ (w_quant * alpha).T) * w_scale * a_scale / 127

Key numerics: a_quant in [-127,127] and w_quant in {-1,0,1} are exactly
representable in bf16; products are integers <= 127 and row sums < 2^24, so a
bf16 PE matmul with fp32 PSUM accumulation is bit-exact.  Rounding to
nearest-even is the (v + 1.5*2^23) - 1.5*2^23 trick in fp32.

Sharding: 2 token groups x 4 out_feature groups across 8 cores.  Per core:
x [4096, 2048], w [2048, 2048], alpha [2048], out [4096, 2048].

Schedule (v2): the PE runs GEMM matmuls essentially exclusively.  All
128x2048 bf16 transposes (quantized weights and activations) go through the
DMA XBAR transpose engine instead of PE identity matmuls.  Both transposes
use the same k permutation, so the contraction is unaffected.  The GEMM
starts as soon as the first 4 weight tiles (out-slice 0) are quantized
(~15us in); the remaining 12 weight tiles stream in under the first
8-block chunk.  y is written back per 512-column slice straight from the
rescale pipeline (no SBUF assembly)."""

import numpy as np

P = 128
K = 2048
TOK = 8192
OUT = 8192
TG, OG = 2, 4
T_LOC = TOK // TG   # 4096
O_LOC = OUT // OG   # 2048
KT = K // P         # 16
NBLK = T_LOC // P   # 32
WT = O_LOC // P     # 16
NSL = O_LOC // 512  # 4
EPS = 1e-8
MAGIC = 12582912.0  # 1.5 * 2^23

_CACHE: dict = {}


def _build_nc():
    import concourse.bacc as bacc
    import concourse.mybir as mybir
    from concourse.tile import TileContext
    from concourse.masks import make_identity

    f32 = mybir.dt.float32
    bf16 = mybir.dt.bfloat16
    ALU = mybir.AluOpType
    ACTF = mybir.ActivationFunctionType
    AX = mybir.AxisListType

    nc = bacc.Bacc("TRN2", target_bir_lowering=False, debug=False, num_devices=8)
    x_d = nc.dram_tensor("x", [T_LOC, K], f32, kind="ExternalInput").ap()
    w_d = nc.dram_tensor("w", [O_LOC, K], f32, kind="ExternalInput").ap()
    al_d = nc.dram_tensor("alpha", [1, O_LOC], f32, kind="ExternalInput").ap()
    y_d = nc.dram_tensor("y", [T_LOC, O_LOC], f32, kind="ExternalOutput").ap()

    with TileContext(nc) as tc:
        with (
            tc.tile_pool(name="singles", bufs=1) as singles,
            tc.tile_pool(name="iopool", bufs=3) as iopool,
            tc.tile_pool(name="scratch", bufs=2) as scratch,
            tc.tile_pool(name="qpool", bufs=2) as qpool,
            tc.tile_pool(name="aqtpool", bufs=12) as aqtpool,
            tc.tile_pool(name="wsmall", bufs=2) as wsmall,
            tc.tile_pool(name="qsmall", bufs=16) as qsmall,
            tc.tile_pool(name="ypool", bufs=6) as ypool,
            tc.tile_pool(name="tppool", bufs=3, space="PSUM") as tppool,
            tc.tile_pool(name="yppool", bufs=5, space="PSUM") as yppool,
        ):
            ident_f32 = singles.tile([P, P], f32)
            make_identity(nc, ident_f32)
            ident_bf = singles.tile([P, P], bf16)
            make_identity(nc, ident_bf)

            w_qT = singles.tile([P, KT, O_LOC], bf16)   # [k-part, k-chunk, out]
            so_bcast = singles.tile([P, O_LOC], f32)
            so_row = singles.tile([1, O_LOC], f32)
            alpha_row = singles.tile([1, O_LOC], f32)
            nc.sync.dma_start(alpha_row, al_d)

            def emit_w_tile(i):
                w_tile = iopool.tile([P, K], f32, tag="in_f32", name="w_tile")
                nc.sync.dma_start(w_tile, w_d[i * P : (i + 1) * P, :])
                # two-stage |W| row sum (close to jnp pairwise summation)
                r1 = wsmall.tile([P, KT], f32, tag="r1", name="r1")
                nc.vector.tensor_reduce(
                    out=r1,
                    in_=w_tile.rearrange("p (a b) -> p a b", b=P),
                    axis=AX.X,
                    op=ALU.add,
                    apply_absolute_value=True,
                )
                ws = wsmall.tile([P, 1], f32, tag="ws", name="ws")
                nc.vector.tensor_reduce(out=ws, in_=r1, axis=AX.X, op=ALU.add)
                nc.vector.tensor_scalar(
                    out=ws, in0=ws, scalar1=1.0 / K, scalar2=EPS,
                    op0=ALU.mult, op1=ALU.add,
                )
                inv_ws = wsmall.tile([P, 1], f32, tag="inv_ws", name="inv_ws")
                nc.vector.reciprocal(inv_ws, ws)
                # ws row entry for rescale: [P,1] -> [1,P] on PE (fp32)
                tpr = tppool.tile([P, 4, P], f32, tag="tp", name="tpr")
                nc.tensor.matmul(
                    tpr[0:1, 0, :], lhsT=ws, rhs=ident_f32, start=True, stop=True
                )
                nc.vector.tensor_copy(
                    so_row[0:1, i * P : (i + 1) * P], tpr[0:1, 0, :]
                )
                # round(W/ws): t1 = W*inv_ws + MAGIC, -MAGIC (ACT); clip (DVE)
                t1 = scratch.tile([P, K], f32, tag="scr", name="t1")
                nc.scalar.activation(
                    t1, w_tile, ACTF.Copy, bias=MAGIC, scale=inv_ws
                )
                nc.scalar.activation(t1, t1, ACTF.Copy, bias=-MAGIC, scale=1.0)
                wq = qpool.tile([P, K], bf16, tag="qb", name="wq")
                nc.vector.tensor_scalar(
                    out=wq, in0=t1, scalar1=1.0, scalar2=-1.0,
                    op0=ALU.min, op1=ALU.max,
                )
                # transpose 16 [128,128] chunks on PE (identity matmuls)
                for g in range(4):
                    tp = tppool.tile([P, 4, P], f32, tag="tp", name="tp")
                    for jj in range(4):
                        j = 4 * g + jj
                        nc.tensor.matmul(
                            tp[:, jj, :],
                            lhsT=wq[:, j * P : (j + 1) * P],
                            rhs=ident_bf,
                            start=True, stop=True,
                        )
                    nc.scalar.copy(w_qT[:, 4 * g : 4 * g + 4, i * P : (i + 1) * P], tp)

            def emit_so_slice(ni):
                sl = slice(ni * 512, (ni + 1) * 512)
                so_tmp = wsmall.tile([1, 512], f32, tag="so_tmp", name="so_tmp")
                nc.vector.tensor_tensor(
                    out=so_tmp, in0=so_row[0:1, sl], in1=alpha_row[0:1, sl],
                    op=ALU.mult,
                )
                nc.gpsimd.partition_broadcast(so_bcast[:, sl], so_tmp)

            def emit_quant(b):
                x_tile = iopool.tile([P, K], f32, tag="in_f32", name="x_tile")
                nc.sync.dma_start(x_tile, x_d[b * P : (b + 1) * P, :])
                amax = qsmall.tile([P, 1], f32, tag="amax", name="amax", bufs=3)
                nc.vector.tensor_reduce(
                    out=amax, in_=x_tile, axis=AX.X, op=ALU.max,
                    apply_absolute_value=True,
                )
                ascale = qsmall.tile([P, 1], f32, tag="ascale", name="ascale", bufs=3)
                nc.vector.tensor_scalar_add(ascale, amax, EPS)
                inv = qsmall.tile([P, 1], f32, tag="inv", name="inv", bufs=3)
                nc.vector.reciprocal(inv, ascale)
                inv127 = qsmall.tile([P, 1], f32, tag="inv127", name="inv127", bufs=3)
                nc.vector.tensor_scalar_mul(inv127, inv, 127.0)
                s_t = qsmall.tile([P, 1], f32, tag="s_t", name="s_t")
                nc.vector.tensor_scalar_mul(s_t, ascale, 1.0 / 127.0)
                t_a = scratch.tile([P, K], f32, tag="scr", name="t_a")
                nc.vector.tensor_scalar(
                    out=t_a, in0=x_tile, scalar1=inv127, scalar2=MAGIC,
                    op0=ALU.mult, op1=ALU.add,
                )
                a_q = qpool.tile([P, K], bf16, tag="qb", name="a_q")
                nc.scalar.activation(a_q, t_a, ACTF.Copy, bias=-MAGIC, scale=1.0)
                a_qT = aqtpool.tile([P, KT, P], bf16, tag="a_qT", name="a_qT")
                for g in range(4):
                    tp = tppool.tile([P, 4, P], f32, tag="tp", name="tpq")
                    for jj in range(4):
                        j = 4 * g + jj
                        nc.tensor.matmul(
                            tp[:, jj, :],
                            lhsT=a_q[:, j * P : (j + 1) * P],
                            rhs=ident_bf,
                            start=True, stop=True,
                        )
                    dst = a_qT[:, 4 * g : 4 * g + 4, :]
                    if g % 2 == 0:
                        nc.vector.tensor_copy(dst, tp)
                    else:
                        nc.scalar.copy(dst, tp)
                return a_qT, s_t

            # ---------- prologue: out-slice 0 weights + first blocks --------
            blk = {}
            emit_w_tile(0)
            emit_w_tile(1)
            blk[0] = emit_quant(0)
            emit_w_tile(2)
            emit_w_tile(3)
            emit_so_slice(0)
            blk[1] = emit_quant(1)

            def emit_gemm(b, n):
                a_qT, s_t = blk[b]
                yp = yppool.tile([P, 512], f32, tag="yp", name="yp")
                for j in range(KT):
                    nc.tensor.matmul(
                        yp,
                        lhsT=a_qT[:, j, :],
                        rhs=w_qT[:, j, n * 512 : (n + 1) * 512],
                        start=(j == 0),
                        stop=(j == KT - 1),
                    )
                ysl = ypool.tile([P, 512], f32, tag="y_sb", name="y_sb")
                nc.scalar.activation(ysl, yp, ACTF.Copy, bias=0.0, scale=s_t)
                nc.vector.tensor_tensor(
                    out=ysl, in0=ysl,
                    in1=so_bcast[:, n * 512 : (n + 1) * 512],
                    op=ALU.mult,
                )
                nc.sync.dma_start(
                    y_d[b * P : (b + 1) * P, n * 512 : (n + 1) * 512], ysl
                )

            # chunk 0: 8 blocks, "diagonal" (block, slice) order chosen so
            # GEMM consumption tracks DMA arrival (x blocks at ~3 us each,
            # w out-slices at ~12 us each).  One background emission per
            # slot sets the DMA queue order; greedy order precomputed from
            # land-time estimates.
            c0_sched = [
                (0, 0), (1, 0), (2, 0), (3, 0), (0, 1), (1, 1), (2, 1),
                (4, 0), (5, 0), (3, 1), (6, 0), (4, 1), (7, 0), (5, 1),
                (6, 1), (7, 1), (0, 2), (1, 2), (2, 2), (3, 2), (4, 2),
                (5, 2), (6, 2), (7, 2), (0, 3), (1, 3), (2, 3), (3, 3),
                (4, 3), (5, 3), (6, 3), (7, 3),
            ]
            c0_bg = [
                "q2", "w4", "q3", "w5", "w6", "w7", "q4", "w8", "q5", "w9",
                "q6", "w10", "q7", "w11", "w12", "w13", "w14", "w15",
                "q8", None, "q9", None, "q10", None, "q11",
            ]
            for si, (b, n) in enumerate(c0_sched):
                item = c0_bg[si] if si < len(c0_bg) else None
                if item is not None:
                    if item[0] == "q":
                        qb = int(item[1:])
                        blk[qb] = emit_quant(qb)
                    else:
                        wi = int(item[1:])
                        emit_w_tile(wi)
                        if wi % 4 == 3:
                            emit_so_slice(wi // 4)
                assert b in blk, (0, n, b)
                emit_gemm(b, n)
            for b in range(8):
                del blk[b]
            next_q = 12

            # chunks 1..6: 4 blocks each, slice-major waves.  The next
            # chunk's four quants are prefetched two-per-wave in the first
            # two waves (~1.5 chunks of lead) so a_q is always ready when
            # the PE's static schedule reaches the transpose matmuls.
            for ci, s in enumerate(range(8, NBLK, 4)):
                blocks = list(range(s, s + 4))
                for n in range(NSL):
                    for _ in range(2 if n < 2 else 0):
                        if next_q < NBLK:
                            blk[next_q] = emit_quant(next_q)
                            next_q += 1
                    for b in blocks:
                        assert b in blk, (ci, n, b)
                        emit_gemm(b, n)
                for b in blocks:
                    del blk[b]

    nc.compile()
    return nc


def _get_nc():
    if "nc" not in _CACHE:
        _CACHE["nc"] = _build_nc()
    return _CACHE["nc"]


def make_in_maps(x, weight, alpha):
    x = np.ascontiguousarray(np.asarray(x, dtype=np.float32).reshape(TOK, K))
    w = np.ascontiguousarray(np.asarray(weight, dtype=np.float32))
    al = np.ascontiguousarray(np.asarray(alpha, dtype=np.float32))
    in_maps = []
    for c in range(TG * OG):
        tg, og = divmod(c, OG)
        in_maps.append(
            {
                "x": np.ascontiguousarray(x[tg * T_LOC : (tg + 1) * T_LOC]),
                "w": np.ascontiguousarray(w[og * O_LOC : (og + 1) * O_LOC]),
                "alpha": np.ascontiguousarray(
                    al[og * O_LOC : (og + 1) * O_LOC].reshape(1, O_LOC)
                ),
            }
        )
    return in_maps


def assemble(results):
    out = np.empty((TOK, OUT), dtype=np.float32)
    for c in range(TG * OG):
        tg, og = divmod(c, OG)
        out[tg * T_LOC : (tg + 1) * T_LOC, og * O_LOC : (og + 1) * O_LOC] = results[
            c
        ]["y"]
    return out.reshape(TG, T_LOC, OUT)


def kernel(x, weight, alpha, _trace=False, **_trace_kwargs):
    from concourse.bass_utils import run_bass_kernel_spmd

    nc = _get_nc()
    in_maps = make_in_maps(x, weight, alpha)
    res = run_bass_kernel_spmd(
        nc, in_maps, core_ids=list(range(TG * OG)), trace=_trace, **_trace_kwargs
    )
    _CACHE["last_results"] = res
    return assemble(res.results)


# revision 26
# speedup vs baseline: 1.0265x; 1.0265x over previous
"""BitLinear (ternary-weight / int8-activation quantized linear) on 8 TRN2 NeuronCores.

Computation (matches reference):
    w_scale = mean(|W|, axis=in) + eps            # [out, 1]
    w_quant = clip(round(W / w_scale), -1, 1)     # ternary
    a_scale = max(|x|, axis=in) + eps             # per token
    a_quant = round(x / a_scale * 127)            # int8 range
    y       = (a_quant @ (w_quant * alpha).T) * w_scale * a_scale / 127

Key numerics: a_quant in [-127,127] and w_quant in {-1,0,1} are exactly
representable in bf16; products are integers <= 127 and row sums < 2^24, so a
bf16 PE matmul with fp32 PSUM accumulation is bit-exact.  Rounding to
nearest-even is the (v + 1.5*2^23) - 1.5*2^23 trick in fp32.

Sharding: 2 token groups x 4 out_feature groups across 8 cores.  Per core:
x [4096, 2048], w [2048, 2048], alpha [2048], out [4096, 2048].

Schedule: the GEMM starts ~15us in, as soon as weight out-slice 0 (tiles
0-3) is quantized; chunk 0 covers 8 token blocks in a "diagonal"
(block, slice) order matched to DMA arrival so the PE never waits long for
weights or activations, while weight tiles 4-15 stream in underneath.
Later chunks run 4 blocks slice-major with activation quantization
prefetched a full chunk ahead.  Transposes are PE identity matmuls
(measured cheaper than the DMA XBAR path, which explodes into per-16-row
descriptors); y is written back per 512-column slice straight from the
rescale pipeline (no SBUF assembly)."""

import numpy as np

P = 128
K = 2048
TOK = 8192
OUT = 8192
TG, OG = 2, 4
T_LOC = TOK // TG   # 4096
O_LOC = OUT // OG   # 2048
KT = K // P         # 16
NBLK = T_LOC // P   # 32
WT = O_LOC // P     # 16
NSL = O_LOC // 512  # 4
EPS = 1e-8
MAGIC = 12582912.0  # 1.5 * 2^23

_CACHE: dict = {}


def _build_nc():
    import concourse.bacc as bacc
    import concourse.mybir as mybir
    from concourse.tile import TileContext
    from concourse.masks import make_identity

    f32 = mybir.dt.float32
    bf16 = mybir.dt.bfloat16
    ALU = mybir.AluOpType
    ACTF = mybir.ActivationFunctionType
    AX = mybir.AxisListType

    nc = bacc.Bacc("TRN2", target_bir_lowering=False, debug=False, num_devices=8)
    x_d = nc.dram_tensor("x", [T_LOC, K], f32, kind="ExternalInput").ap()
    w_d = nc.dram_tensor("w", [O_LOC, K], f32, kind="ExternalInput").ap()
    al_d = nc.dram_tensor("alpha", [1, O_LOC], f32, kind="ExternalInput").ap()
    y_d = nc.dram_tensor("y", [T_LOC, O_LOC], f32, kind="ExternalOutput").ap()

    with TileContext(nc) as tc:
        with (
            tc.tile_pool(name="singles", bufs=1) as singles,
            tc.tile_pool(name="iopool", bufs=3) as iopool,
            tc.tile_pool(name="scratch", bufs=2) as scratch,
            tc.tile_pool(name="qpool", bufs=2) as qpool,
            tc.tile_pool(name="aqtpool", bufs=12) as aqtpool,
            tc.tile_pool(name="wsmall", bufs=2) as wsmall,
            tc.tile_pool(name="qsmall", bufs=16) as qsmall,
            tc.tile_pool(name="ypool", bufs=6) as ypool,
            tc.tile_pool(name="tppool", bufs=2, space="PSUM") as tppool,
            tc.tile_pool(name="yppool", bufs=6, space="PSUM") as yppool,
        ):
            ident_f32 = singles.tile([P, P], f32)
            make_identity(nc, ident_f32)
            ident_bf = singles.tile([P, P], bf16)
            make_identity(nc, ident_bf)

            w_qT = singles.tile([P, KT, O_LOC], bf16)   # [k-part, k-chunk, out]
            so_bcast = singles.tile([P, O_LOC], f32)
            so_row = singles.tile([1, O_LOC], f32)
            alpha_row = singles.tile([1, O_LOC], f32)
            nc.sync.dma_start(alpha_row, al_d)

            def emit_w_tile(i):
                w_tile = iopool.tile([P, K], f32, tag="in_f32", name="w_tile")
                nc.sync.dma_start(w_tile, w_d[i * P : (i + 1) * P, :])
                # two-stage |W| row sum (close to jnp pairwise summation)
                r1 = wsmall.tile([P, KT], f32, tag="r1", name="r1")
                nc.vector.tensor_reduce(
                    out=r1,
                    in_=w_tile.rearrange("p (a b) -> p a b", b=P),
                    axis=AX.X,
                    op=ALU.add,
                    apply_absolute_value=True,
                )
                ws = wsmall.tile([P, 1], f32, tag="ws", name="ws")
                nc.vector.tensor_reduce(out=ws, in_=r1, axis=AX.X, op=ALU.add)
                nc.vector.tensor_scalar(
                    out=ws, in0=ws, scalar1=1.0 / K, scalar2=EPS,
                    op0=ALU.mult, op1=ALU.add,
                )
                inv_ws = wsmall.tile([P, 1], f32, tag="inv_ws", name="inv_ws")
                nc.vector.reciprocal(inv_ws, ws)
                # ws row entry for rescale: [P,1] -> [1,P] on PE (fp32)
                tpr = tppool.tile([P, 4, P], f32, tag="tp", name="tpr")
                nc.tensor.matmul(
                    tpr[0:1, 0, :], lhsT=ws, rhs=ident_f32, start=True, stop=True
                )
                nc.vector.tensor_copy(
                    so_row[0:1, i * P : (i + 1) * P], tpr[0:1, 0, :]
                )
                # round(W/ws): t1 = W*inv_ws + MAGIC, -MAGIC (ACT); clip (DVE)
                t1 = scratch.tile([P, K], f32, tag="scr", name="t1")
                nc.scalar.activation(
                    t1, w_tile, ACTF.Copy, bias=MAGIC, scale=inv_ws
                )
                nc.scalar.activation(t1, t1, ACTF.Copy, bias=-MAGIC, scale=1.0)
                wq = qpool.tile([P, K], bf16, tag="qb", name="wq")
                nc.vector.tensor_scalar(
                    out=wq, in0=t1, scalar1=1.0, scalar2=-1.0,
                    op0=ALU.min, op1=ALU.max,
                )
                # transpose 16 [128,128] chunks on PE (identity matmuls)
                for g in range(4):
                    tp = tppool.tile([P, 4, P], f32, tag="tp", name="tp")
                    for jj in range(4):
                        j = 4 * g + jj
                        nc.tensor.matmul(
                            tp[:, jj, :],
                            lhsT=wq[:, j * P : (j + 1) * P],
                            rhs=ident_bf,
                            start=True, stop=True,
                        )
                    nc.scalar.copy(w_qT[:, 4 * g : 4 * g + 4, i * P : (i + 1) * P], tp)

            def emit_so_slice(ni):
                sl = slice(ni * 512, (ni + 1) * 512)
                so_tmp = wsmall.tile([1, 512], f32, tag="so_tmp", name="so_tmp")
                nc.vector.tensor_tensor(
                    out=so_tmp, in0=so_row[0:1, sl], in1=alpha_row[0:1, sl],
                    op=ALU.mult,
                )
                nc.gpsimd.partition_broadcast(so_bcast[:, sl], so_tmp)

            def emit_quant(b):
                x_tile = iopool.tile([P, K], f32, tag="in_f32", name="x_tile")
                nc.sync.dma_start(x_tile, x_d[b * P : (b + 1) * P, :])
                amax = qsmall.tile([P, 1], f32, tag="amax", name="amax", bufs=3)
                nc.vector.tensor_reduce(
                    out=amax, in_=x_tile, axis=AX.X, op=ALU.max,
                    apply_absolute_value=True,
                )
                ascale = qsmall.tile([P, 1], f32, tag="ascale", name="ascale", bufs=3)
                nc.vector.tensor_scalar_add(ascale, amax, EPS)
                inv = qsmall.tile([P, 1], f32, tag="inv", name="inv", bufs=3)
                nc.vector.reciprocal(inv, ascale)
                inv127 = qsmall.tile([P, 1], f32, tag="inv127", name="inv127", bufs=3)
                nc.vector.tensor_scalar_mul(inv127, inv, 127.0)
                s_t = qsmall.tile([P, 1], f32, tag="s_t", name="s_t")
                nc.vector.tensor_scalar_mul(s_t, ascale, 1.0 / 127.0)
                t_a = scratch.tile([P, K], f32, tag="scr", name="t_a")
                nc.vector.tensor_scalar(
                    out=t_a, in0=x_tile, scalar1=inv127, scalar2=MAGIC,
                    op0=ALU.mult, op1=ALU.add,
                )
                a_q = qpool.tile([P, K], bf16, tag="qb", name="a_q")
                nc.scalar.activation(a_q, t_a, ACTF.Copy, bias=-MAGIC, scale=1.0)
                a_qT = aqtpool.tile([P, KT, P], bf16, tag="a_qT", name="a_qT")
                for g in range(4):
                    tp = tppool.tile([P, 4, P], f32, tag="tp", name="tpq")
                    for jj in range(4):
                        j = 4 * g + jj
                        nc.tensor.matmul(
                            tp[:, jj, :],
                            lhsT=a_q[:, j * P : (j + 1) * P],
                            rhs=ident_bf,
                            start=True, stop=True,
                        )
                    dst = a_qT[:, 4 * g : 4 * g + 4, :]
                    if g % 2 == 0:
                        nc.vector.tensor_copy(dst, tp)
                    else:
                        nc.scalar.copy(dst, tp)
                return a_qT, s_t

            # ---------- prologue: out-slice 0 weights + first blocks --------
            blk = {}
            emit_w_tile(0)
            emit_w_tile(1)
            blk[0] = emit_quant(0)
            emit_w_tile(2)
            emit_w_tile(3)
            emit_so_slice(0)
            blk[1] = emit_quant(1)

            def emit_gemm(b, n):
                a_qT, s_t = blk[b]
                yp = yppool.tile([P, 512], f32, tag="yp", name="yp")
                for j in range(KT):
                    nc.tensor.matmul(
                        yp,
                        lhsT=a_qT[:, j, :],
                        rhs=w_qT[:, j, n * 512 : (n + 1) * 512],
                        start=(j == 0),
                        stop=(j == KT - 1),
                    )
                ysl = ypool.tile([P, 512], f32, tag="y_sb", name="y_sb")
                nc.scalar.activation(ysl, yp, ACTF.Copy, bias=0.0, scale=s_t)
                nc.vector.tensor_tensor(
                    out=ysl, in0=ysl,
                    in1=so_bcast[:, n * 512 : (n + 1) * 512],
                    op=ALU.mult,
                )
                nc.sync.dma_start(
                    y_d[b * P : (b + 1) * P, n * 512 : (n + 1) * 512], ysl
                )

            # chunk 0: 8 blocks, "diagonal" (block, slice) order chosen so
            # GEMM consumption tracks DMA arrival (x blocks at ~3 us each,
            # w out-slices at ~12 us each).  One background emission per
            # slot sets the DMA queue order; greedy order precomputed from
            # land-time estimates.
            c0_sched = [
                (0, 0), (1, 0), (2, 0), (3, 0), (0, 1), (1, 1), (2, 1),
                (4, 0), (5, 0), (3, 1), (6, 0), (4, 1), (7, 0), (5, 1),
                (6, 1), (7, 1), (0, 2), (1, 2), (2, 2), (3, 2), (4, 2),
                (5, 2), (6, 2), (7, 2), (0, 3), (1, 3), (2, 3), (3, 3),
                (4, 3), (5, 3), (6, 3), (7, 3),
            ]
            c0_bg = [
                "q2", "w4", "q3", "w5", "w6", "w7", "q4", "w8", "q5", "w9",
                "q6", "w10", "q7", "w11", "w12", "w13", "w14", "w15",
                "q8", None, "q9", None, "q10", None, "q11",
            ]
            for si, (b, n) in enumerate(c0_sched):
                item = c0_bg[si] if si < len(c0_bg) else None
                if item is not None:
                    if item[0] == "q":
                        qb = int(item[1:])
                        blk[qb] = emit_quant(qb)
                    else:
                        wi = int(item[1:])
                        emit_w_tile(wi)
                        if wi % 4 == 3:
                            emit_so_slice(wi // 4)
                assert b in blk, (0, n, b)
                emit_gemm(b, n)
            for b in range(8):
                del blk[b]
            next_q = 12

            # chunks 1..6: 4 blocks each, slice-major waves, one quant
            # prefetched at each wave start (one full chunk of lead).
            for ci, s in enumerate(range(8, NBLK, 4)):
                blocks = list(range(s, s + 4))
                for n in range(NSL):
                    if next_q < NBLK:
                        blk[next_q] = emit_quant(next_q)
                        next_q += 1
                    for b in blocks:
                        assert b in blk, (ci, n, b)
                        emit_gemm(b, n)
                for b in blocks:
                    del blk[b]

    nc.compile()
    return nc


def _get_nc():
    if "nc" not in _CACHE:
        _CACHE["nc"] = _build_nc()
    return _CACHE["nc"]


def make_in_maps(x, weight, alpha):
    x = np.ascontiguousarray(np.asarray(x, dtype=np.float32).reshape(TOK, K))
    w = np.ascontiguousarray(np.asarray(weight, dtype=np.float32))
    al = np.ascontiguousarray(np.asarray(alpha, dtype=np.float32))
    in_maps = []
    for c in range(TG * OG):
        tg, og = divmod(c, OG)
        in_maps.append(
            {
                "x": np.ascontiguousarray(x[tg * T_LOC : (tg + 1) * T_LOC]),
                "w": np.ascontiguousarray(w[og * O_LOC : (og + 1) * O_LOC]),
                "alpha": np.ascontiguousarray(
                    al[og * O_LOC : (og + 1) * O_LOC].reshape(1, O_LOC)
                ),
            }
        )
    return in_maps


def assemble(results):
    out = np.empty((TOK, OUT), dtype=np.float32)
    for c in range(TG * OG):
        tg, og = divmod(c, OG)
        out[tg * T_LOC : (tg + 1) * T_LOC, og * O_LOC : (og + 1) * O_LOC] = results[
            c
        ]["y"]
    return out.reshape(TG, T_LOC, OUT)


def kernel(x, weight, alpha, _trace=False, **_trace_kwargs):
    from concourse.bass_utils import run_bass_kernel_spmd

    nc = _get_nc()
    in_maps = make_in_maps(x, weight, alpha)
    res = run_bass_kernel_spmd(
        nc, in_maps, core_ids=list(range(TG * OG)), trace=_trace, **_trace_kwargs
    )
    _CACHE["last_results"] = res
    return assemble(res.results)
